# revision 2
# baseline (speedup 1.0000x reference)
"""Trainium2 Bass kernel for nn_CustomLSTM (T=512, B=64, I=H=1024).

Strategy: data-parallel over batch (8 cores x 8 rows each). Per core:
  Phase 1: xg = x @ w_ih.T + (b_ih + b_hh)    -- batched matmul, fp32r
  Phase 2: 512-step recurrence:
     gates_t = xg_t + h_{t-1} @ w_hh.T   (PE: identity-selector preload of
               xg into PSUM, then 8 K-tile accumulation matmuls, fp32r)
     i,f,g,o = act(gates)                 (ScalarE, direct from PSUM)
     c = f*c + i*g ; h = o*tanh(c)        (VectorE)
     hT tiles via PE transpose            (stationary operand for next step)

Gate columns are host-permuted to [f | i | g | o] blocks. Weights/x are
host-transposed (pure layout work; all FLOPs run on device).
"""

import os
import sys

sys.path.insert(0, "/opt/trn_rl_repo")

import numpy as np

import concourse.bass as bass
import concourse.mybir as mybir
import concourse.tile as tile
from concourse import bacc
from concourse.bass import ds, ts
from concourse.bass_utils import run_bass_kernel_spmd

fp32 = mybir.dt.float32
fp32r = mybir.dt.float32r
AF = mybir.ActivationFunctionType
ALU = mybir.AluOpType

T, B, I, H = 512, 64, 1024, 1024
NCORES = 8
BL = B // NCORES          # 8 batch rows per core
G4 = 4 * H                # 4096 gate columns
KT = I // 128             # 8 contraction tiles
NCH = G4 // 512           # 8 psum chunks per step
HT = H // 128             # 8 hidden tiles


def build_nc(t_steps: int):
    nc = bacc.Bacc("TRN2", target_bir_lowering=False)

    xT = nc.dram_tensor("xT", [I, t_steps * BL], fp32r, kind="ExternalInput")
    wihT = nc.dram_tensor("wihT", [I, G4], fp32r, kind="ExternalInput")
    whhT = nc.dram_tensor("whhT", [I, G4], fp32r, kind="ExternalInput")
    biasr = nc.dram_tensor("biasr", [128, G4], fp32, kind="ExternalInput")
    h0T = nc.dram_tensor("h0T", [H, BL], fp32r, kind="ExternalInput")
    c0 = nc.dram_tensor("c0", [BL, H], fp32, kind="ExternalInput")
    ident = nc.dram_tensor("ident", [128, 128], fp32r, kind="ExternalInput")

    out_h = nc.dram_tensor("out_h", [t_steps, BL, H], fp32, kind="ExternalOutput")
    out_i = nc.dram_tensor("out_i", [t_steps, BL, H], fp32, kind="ExternalOutput")
    out_f = nc.dram_tensor("out_f", [t_steps, BL, H], fp32, kind="ExternalOutput")
    out_g = nc.dram_tensor("out_g", [t_steps, BL, H], fp32, kind="ExternalOutput")
    out_o = nc.dram_tensor("out_o", [t_steps, BL, H], fp32, kind="ExternalOutput")
    out_c = nc.dram_tensor("out_c", [BL, H], fp32, kind="ExternalOutput")

    n_blocks = (t_steps + 15) // 16

    with tile.TileContext(nc) as tc:
        with (
            tc.tile_pool(name="wpool", bufs=1) as wpool,
            tc.tile_pool(name="cstp", bufs=1) as cstp,
            tc.tile_pool(name="dramp", bufs=1, space="DRAM") as dramp,
        ):
            i128 = cstp.tile([128, 128], fp32r, tag="ident")
            nc.sync.dma_start(i128[:], ident[:])

            xg_dram = dramp.tile([t_steps * BL, G4], fp32r, tag="xg")

            # ---------------- Phase 1: xg = x @ wihT + bias ----------------
            wih_sb = wpool.tile([128, KT, G4], fp32r, tag="wbig", name="wih_sb")
            nc.sync.dma_start(
                wih_sb[:], wihT[:].rearrange("(ko p) g -> p ko g", p=128)
            )
            xT_r = xT[:].rearrange("(ko p) r -> p ko r", p=128)

            with (
                tc.tile_pool(name="p1x", bufs=3) as p1x,
                tc.tile_pool(name="p1b", bufs=1) as p1b,
                tc.tile_pool(name="p1e", bufs=4) as p1e,
                tc.tile_pool(name="ps1", bufs=4, space="PSUM") as ps1,
            ):
                bias_sb = p1b.tile([128, G4], fp32, tag="bias")
                nc.sync.dma_start(bias_sb[:], biasr[:])

                n_mtiles = t_steps * BL // 128
                for m in range(n_mtiles):
                    xt = p1x.tile([128, KT, 128], fp32r, tag="xt")
                    nc.sync.dma_start(xt[:], xT_r[:, :, ds(128 * m, 128)])
                    for n in range(NCH):
                        psx = ps1.tile([128, 512], fp32, tag="px")
                        for ko in range(KT):
                            nc.tensor.matmul(
                                psx[:],
                                xt[:, ko, :],
                                wih_sb[:, ko, ds(512 * n, 512)],
                                start=(ko == 0),
                                stop=(ko == KT - 1),
                            )
                        ev = p1e.tile([128, 512], fp32r, tag="ev")
                        nc.vector.tensor_tensor(
                            ev[:], psx[:], bias_sb[:, ds(512 * n, 512)], ALU.add
                        )
                        nc.sync.dma_start(
                            xg_dram[ds(128 * m, 128), ds(512 * n, 512)], ev[:]
                        )

            # ---------------- Phase 2: recurrence ----------------
            whh_sb = wpool.tile([128, KT, G4], fp32r, tag="wbig", name="whh_sb")
            nc.sync.dma_start(
                whh_sb[:], whhT[:].rearrange("(ko p) g -> p ko g", p=128)
            )

            with (
                tc.tile_pool(name="xgp", bufs=2) as xgp,
                tc.tile_pool(name="htp", bufs=2) as htp,
                tc.tile_pool(name="cp", bufs=2) as cp,
                tc.tile_pool(name="gp", bufs=1) as gp,
                tc.tile_pool(name="hp", bufs=1) as hp,
                tc.tile_pool(name="tcp", bufs=1) as tcp,
                tc.tile_pool(name="psg", bufs=5, space="PSUM") as psg,
                tc.tile_pool(name="pst", bufs=2, space="PSUM") as pst,
            ):
                hT_cur = htp.tile([128, KT, BL], fp32r, tag="hT", name="hT_init")
                nc.sync.dma_start(
                    hT_cur[:], h0T[:].rearrange("(ko p) b -> p ko b", p=128)
                )
                c_cur = cp.tile([BL, H], fp32, tag="c", name="c_init")
                nc.sync.dma_start(c_cur[:], c0[:])

                xg_sb = None
                for t in range(t_steps):
                    blk, s = divmod(t, 16)
                    if s == 0:
                        xg_sb = xgp.tile([128, G4], fp32r, tag="xg")
                        nrows = min(128, t_steps * BL - 128 * blk)
                        nc.sync.dma_start(
                            xg_sb[:nrows], xg_dram[ds(128 * blk, nrows), :]
                        )

                    # gate tiles for this step (packed order f,i,g,o)
                    f_sb = gp.tile([BL, H], fp32, tag="fsb")
                    i_sb = gp.tile([BL, H], fp32, tag="isb")
                    g_sb = gp.tile([BL, H], fp32, tag="gsb")
                    o_sb = gp.tile([BL, H], fp32, tag="osb")
                    gtiles = [f_sb, i_sb, g_sb, o_sb]
                    gfuncs = [AF.Sigmoid, AF.Sigmoid, AF.Tanh, AF.Sigmoid]

                    for ch in range(NCH):
                        q, hh = divmod(ch, 2)
                        pg = psg.tile([BL, 512], fp32, tag="pg")
                        nc.tensor.matmul(
                            pg[:],
                            i128[:, ds(8 * s, BL)],
                            xg_sb[:, ds(512 * ch, 512)],
                            start=True,
                            stop=False,
                        )
                        for ko in range(KT):
                            nc.tensor.matmul(
                                pg[:],
                                hT_cur[:, ko, :],
                                whh_sb[:, ko, ds(512 * ch, 512)],
                                start=False,
                                stop=(ko == KT - 1),
                            )
                        nc.scalar.activation(
                            gtiles[q][:, ds(512 * hh, 512)], pg[:], gfuncs[q]
                        )

                    # c_new = f*c + i*g ; h = o*tanh(c_new)
                    c_new = cp.tile([BL, H], fp32, tag="c")
                    tmp = tcp.tile([BL, H], fp32, tag="tmp")
                    nc.vector.tensor_tensor(tmp[:], i_sb[:], g_sb[:], ALU.mult)
                    nc.vector.tensor_tensor(c_new[:], f_sb[:], c_cur[:], ALU.mult)
                    nc.vector.tensor_tensor(c_new[:], c_new[:], tmp[:], ALU.add)
                    tc_sb = tcp.tile([BL, H], fp32, tag="tc")
                    nc.scalar.activation(tc_sb[:], c_new[:], AF.Tanh)
                    h_sb = hp.tile([BL, H], fp32r, tag="h")
                    nc.vector.tensor_tensor(h_sb[:], o_sb[:], tc_sb[:], ALU.mult)

                    # outputs
                    nc.sync.dma_start(out_h[t], h_sb[:].bitcast(fp32))
                    nc.sync.dma_start(out_f[t], f_sb[:])
                    nc.sync.dma_start(out_i[t], i_sb[:])
                    nc.sync.dma_start(out_g[t], g_sb[:])
                    nc.sync.dma_start(out_o[t], o_sb[:])

                    # hT for next step via PE transpose
                    if t < t_steps - 1:
                        hT_new = htp.tile([128, KT, BL], fp32r, tag="hT")
                        for j in range(HT):
                            pt = pst.tile([128, BL], fp32r, tag="pt")
                            nc.tensor.transpose(
                                pt[:], h_sb[:, ds(128 * j, 128)], i128[:BL, :BL]
                            )
                            nc.vector.tensor_copy(hT_new[:, j, :], pt[:])
                        hT_cur = hT_new
                    c_cur = c_new

                nc.sync.dma_start(out_c[:], c_cur[:])

    nc.compile()
    return nc


# gate permutation: packed [f|i|g|o] from reference order [i|f|g|o]
def _gate_perm():
    a = np.arange(H)
    return np.concatenate([H + a, a, 2 * H + a, 3 * H + a])


def _prep_inputs(x, h0, c0, w_ih, w_hh, b_ih, b_hh, t_steps):
    perm = _gate_perm()
    wihT = np.ascontiguousarray(w_ih[perm].T, dtype=np.float32)
    whhT = np.ascontiguousarray(w_hh[perm].T, dtype=np.float32)
    bias = (b_ih + b_hh)[perm].astype(np.float32)
    biasr = np.ascontiguousarray(np.broadcast_to(bias, (128, G4)))
    identity = np.eye(128, dtype=np.float32)

    in_maps = []
    for k in range(NCORES):
        sl = slice(k * BL, (k + 1) * BL)
        xk = x[:t_steps, sl, :]  # [t, BL, I]
        xTk = np.ascontiguousarray(
            xk.transpose(2, 0, 1).reshape(I, t_steps * BL), dtype=np.float32
        )
        h0Tk = np.ascontiguousarray(h0[sl].T, dtype=np.float32)
        c0k = np.ascontiguousarray(c0[sl], dtype=np.float32)
        in_maps.append(
            {
                "xT": xTk,
                "wihT": wihT,
                "whhT": whhT,
                "biasr": biasr,
                "h0T": h0Tk,
                "c0": c0k,
                "ident": identity,
            }
        )
    return in_maps


_NC_CACHE = {}


def run(x, h0, c0, w_ih, w_hh, b_ih, b_hh, t_steps=T, trace=False):
    x = np.asarray(x, dtype=np.float32)
    h0 = np.asarray(h0, dtype=np.float32)
    c0 = np.asarray(c0, dtype=np.float32)
    w_ih = np.asarray(w_ih, dtype=np.float32)
    w_hh = np.asarray(w_hh, dtype=np.float32)
    b_ih = np.asarray(b_ih, dtype=np.float32)
    b_hh = np.asarray(b_hh, dtype=np.float32)

    if t_steps not in _NC_CACHE:
        _NC_CACHE[t_steps] = build_nc(t_steps)
    nc = _NC_CACHE[t_steps]

    in_maps = _prep_inputs(x, h0, c0, w_ih, w_hh, b_ih, b_hh, t_steps)
    res = run_bass_kernel_spmd(
        nc, in_maps, core_ids=list(range(NCORES)), trace=trace
    )

    outs = res.results
    full_h = np.concatenate([outs[k]["out_h"] for k in range(NCORES)], axis=1)
    full_i = np.concatenate([outs[k]["out_i"] for k in range(NCORES)], axis=1)
    full_f = np.concatenate([outs[k]["out_f"] for k in range(NCORES)], axis=1)
    full_g = np.concatenate([outs[k]["out_g"] for k in range(NCORES)], axis=1)
    full_o = np.concatenate([outs[k]["out_o"] for k in range(NCORES)], axis=1)
    c_t = np.concatenate([outs[k]["out_c"] for k in range(NCORES)], axis=0)
    h_t = full_h[-1]
    ret = (full_h, h_t, c_t, full_i, full_f, full_g, full_o)
    return ret, res


def kernel(**inputs):
    ret, _ = run(
        inputs["x"],
        inputs["h0"],
        inputs["c0"],
        inputs["w_ih"],
        inputs["w_hh"],
        inputs["b_ih"],
        inputs["b_hh"],
        t_steps=T,
    )
    return ret


# revision 12
# speedup vs baseline: 850.5395x; 850.5395x over previous
"""Trainium2 Bass kernel for nn_CustomLSTM (T=512, B=64, I=H=1024).

Strategy: data-parallel over batch (8 cores x 8 rows each). Per core:
  Phase 1: xg = x @ w_ih.T + (b_ih + b_hh)    -- batched matmul, fp32r
  Phase 2: 512-step recurrence:
     gates_t = xg_t + h_{t-1} @ w_hh.T   (PE: identity-selector preload of
               xg into PSUM, then 8 K-tile accumulation matmuls, fp32r)
     i,f,g,o = act(gates)                 (ScalarE, direct from PSUM)
     c = f*c + i*g ; h = o*tanh(c)        (VectorE)
     hT tiles via PE transpose            (stationary operand for next step)

Gate columns are host-permuted to [f | i | g | o] blocks. Weights/x are
host-transposed (pure layout work; all FLOPs run on device).
"""

import os
import sys

sys.path.insert(0, "/opt/trn_rl_repo")

import numpy as np

import concourse.bass as bass
import concourse.mybir as mybir
import concourse.tile as tile
from concourse import bacc
from concourse.bass import ds, ts
from concourse.bass_utils import run_bass_kernel_spmd

fp32 = mybir.dt.float32
fp32r = mybir.dt.float32r
AF = mybir.ActivationFunctionType
ALU = mybir.AluOpType

T, B, I, H = 512, 64, 1024, 1024
NCORES = 8
BL = B // NCORES          # 8 batch rows per core
G4 = 4 * H                # 4096 gate columns
KT = I // 128             # 8 contraction tiles
NCH = G4 // 512           # 8 psum chunks per step
HT = H // 128             # 8 hidden tiles


def build_nc(t_steps: int):
    nc = bacc.Bacc("TRN2", target_bir_lowering=False)

    xT = nc.dram_tensor("xT", [I, t_steps * BL], fp32r, kind="ExternalInput")
    wihT = nc.dram_tensor("wihT", [I, G4], fp32r, kind="ExternalInput")
    whhT = nc.dram_tensor("whhT", [I, G4], fp32r, kind="ExternalInput")
    biasr = nc.dram_tensor("biasr", [128, G4], fp32, kind="ExternalInput")
    h0T = nc.dram_tensor("h0T", [H, BL], fp32r, kind="ExternalInput")
    c0 = nc.dram_tensor("c0", [BL, H], fp32, kind="ExternalInput")
    ident = nc.dram_tensor("ident", [128, 128], fp32r, kind="ExternalInput")

    out_h = nc.dram_tensor("out_h", [t_steps, BL, H], fp32, kind="ExternalOutput")
    out_i = nc.dram_tensor("out_i", [t_steps, BL, H], fp32, kind="ExternalOutput")
    out_f = nc.dram_tensor("out_f", [t_steps, BL, H], fp32, kind="ExternalOutput")
    out_g = nc.dram_tensor("out_g", [t_steps, BL, H], fp32, kind="ExternalOutput")
    out_o = nc.dram_tensor("out_o", [t_steps, BL, H], fp32, kind="ExternalOutput")
    out_c = nc.dram_tensor("out_c", [BL, H], fp32, kind="ExternalOutput")

    n_blocks = (t_steps + 15) // 16

    with tile.TileContext(nc) as tc:
        with (
            tc.tile_pool(name="wpool", bufs=1) as wpool,
            tc.tile_pool(name="cstp", bufs=1) as cstp,
            tc.tile_pool(name="dramp", bufs=1, space="DRAM") as dramp,
        ):
            i128 = cstp.tile([128, 128], fp32r, tag="ident")
            nc.sync.dma_start(i128[:], ident[:])

            xg_dram = dramp.tile([t_steps * BL, G4], fp32r, tag="xg")

            # ---------------- Phase 1: xg = x @ wihT + bias ----------------
            wih_sb = wpool.tile([128, KT, G4], fp32r, tag="wbig", name="wih_sb")
            nc.sync.dma_start(
                wih_sb[:], wihT[:].rearrange("(ko p) g -> p ko g", p=128)
            )
            xT_r = xT[:].rearrange("(ko p) r -> p ko r", p=128)

            with (
                tc.tile_pool(name="p1x", bufs=3) as p1x,
                tc.tile_pool(name="p1b", bufs=1) as p1b,
                tc.tile_pool(name="p1e", bufs=4) as p1e,
                tc.tile_pool(name="ps1", bufs=4, space="PSUM") as ps1,
            ):
                bias_sb = p1b.tile([128, G4], fp32, tag="bias")
                nc.sync.dma_start(bias_sb[:], biasr[:])

                n_mtiles = t_steps * BL // 128
                for m in range(n_mtiles):
                    xt = p1x.tile([128, KT, 128], fp32r, tag="xt")
                    nc.sync.dma_start(xt[:], xT_r[:, :, ds(128 * m, 128)])
                    for n in range(NCH):
                        psx = ps1.tile([128, 512], fp32, tag="px")
                        for ko in range(KT):
                            nc.tensor.matmul(
                                psx[:],
                                xt[:, ko, :],
                                wih_sb[:, ko, ds(512 * n, 512)],
                                start=(ko == 0),
                                stop=(ko == KT - 1),
                            )
                        ev = p1e.tile([128, 512], fp32r, tag="ev")
                        nc.vector.tensor_tensor(
                            ev[:], psx[:], bias_sb[:, ds(512 * n, 512)], ALU.add
                        )
                        nc.sync.dma_start(
                            xg_dram[ds(128 * m, 128), ds(512 * n, 512)], ev[:]
                        )

            # ---------------- Phase 2: recurrence ----------------
            whh_sb = wpool.tile([128, KT, G4], fp32r, tag="wbig", name="whh_sb")
            nc.sync.dma_start(
                whh_sb[:], whhT[:].rearrange("(ko p) g -> p ko g", p=128)
            )

            with (
                tc.tile_pool(name="xgp", bufs=2) as xgp,
                tc.tile_pool(name="htp", bufs=2) as htp,
                tc.tile_pool(name="cp", bufs=2) as cp,
                tc.tile_pool(name="gp", bufs=1) as gp,
                tc.tile_pool(name="hp", bufs=1) as hp,
                tc.tile_pool(name="tcp", bufs=1) as tcp,
                tc.tile_pool(name="psg", bufs=6, space="PSUM") as psg,
                tc.tile_pool(name="pst", bufs=2, space="PSUM") as pst,
            ):
                hT_cur = htp.tile([128, KT, BL], fp32r, tag="hT", name="hT_init")
                nc.sync.dma_start(
                    hT_cur[:], h0T[:].rearrange("(ko p) b -> p ko b", p=128)
                )
                c_cur = cp.tile([BL, H], fp32, tag="c", name="c_init")
                nc.sync.dma_start(c_cur[:], c0[:])

                xg_sb = None
                for t in range(t_steps):
                    blk, s = divmod(t, 16)
                    if s == 0:
                        xg_sb = xgp.tile([128, G4], fp32r, tag="xg", name=f"xgb_{blk}")
                        nrows = min(128, t_steps * BL - 128 * blk)
                        nc.sync.dma_start(
                            xg_sb[:nrows], xg_dram[ds(128 * blk, nrows), :]
                        )

                    # gate tiles for this step (packed order f,i,g,o)
                    f_sb = gp.tile([BL, H], fp32, tag="fsb")
                    i_sb = gp.tile([BL, H], fp32, tag="isb")
                    g_sb = gp.tile([BL, H], fp32, tag="gsb")
                    o_sb = gp.tile([BL, H], fp32, tag="osb")
                    c_new = cp.tile([BL, H], fp32, tag="c")
                    tmp = tcp.tile([BL, H], fp32, tag="tmp")
                    tc_sb = tcp.tile([BL, H], fp32, tag="tc")
                    h_sb = hp.tile([BL, H], fp32r, tag="h")
                    hT_new = (
                        htp.tile([128, KT, BL], fp32r, tag="hT", name=f"hT_{t}")
                        if t < t_steps - 1
                        else None
                    )

                    # chunk order: f0 i0 g0 f1 i1 g1 o0 o1 -> half hh of each
                    # gate block; elementwise runs per-half as soon as its
                    # gates are ready, so the h-tail overlaps the o-matmuls
                    # and next-step preloads.
                    CHORD = [(0, 0), (1, 0), (2, 0), (0, 1), (1, 1), (2, 1), (3, 0), (3, 1)]
                    gtiles = [f_sb, i_sb, g_sb, o_sb]
                    gfuncs = [AF.Sigmoid, AF.Sigmoid, AF.Tanh, AF.Sigmoid]

                    pgs = [psg.tile([BL, 512], fp32, tag="pg", name=f"pg_{t}_{k}") for k in range(NCH)]
                    npre = 6

                    def preload(ci):
                        q, hh = CHORD[ci]
                        nc.tensor.matmul(
                            pgs[ci][:],
                            i128[:, ds(8 * s, BL)],
                            xg_sb[:, ds(512 * (2 * q + hh), 512)],
                            start=True,
                            stop=False,
                        )

                    def do_half(hh):
                        # c/h update for hidden cols [512*hh : 512*hh+512]
                        cs = ds(512 * hh, 512)
                        nc.vector.tensor_tensor(tmp[:, cs], i_sb[:, cs], g_sb[:, cs], ALU.mult)
                        nc.vector.tensor_tensor(c_new[:, cs], f_sb[:, cs], c_cur[:, cs], ALU.mult)
                        nc.vector.tensor_tensor(c_new[:, cs], c_new[:, cs], tmp[:, cs], ALU.add)
                        nc.scalar.activation(tc_sb[:, cs], c_new[:, cs], AF.Tanh)

                    pt128 = (
                        pst.tile([128, KT * BL], fp32r, tag="pt", name=f"pt_{t}")
                        if hT_new is not None
                        else None
                    )

                    def do_h_half(hh):
                        cs = ds(512 * hh, 512)
                        nc.vector.tensor_tensor(h_sb[:, cs], o_sb[:, cs], tc_sb[:, cs], ALU.mult)
                        if hT_new is not None:
                            for j in range(4 * hh, 4 * hh + 4):
                                nc.tensor.transpose(
                                    pt128[:, ds(BL * j, BL)],
                                    h_sb[:, ds(128 * j, 128)],
                                    i128[:BL, :BL],
                                )
                            nc.vector.tensor_copy(
                                hT_new[:, ds(4 * hh, 4), :],
                                pt128[:, ds(32 * hh, 32)].rearrange(
                                    "p (j b) -> p j b", b=BL
                                ),
                            )

                    for ci in range(npre):
                        preload(ci)
                    for ci in range(NCH):
                        q, hh = CHORD[ci]
                        pg = pgs[ci]
                        for ko in range(KT):
                            nc.tensor.matmul(
                                pg[:],
                                hT_cur[:, ko, :],
                                whh_sb[:, ko, ds(512 * (2 * q + hh), 512)],
                                start=False,
                                stop=(ko == KT - 1),
                            )
                        nc.scalar.activation(
                            gtiles[q][:, ds(512 * hh, 512)], pg[:], gfuncs[q]
                        )
                        if ci + npre < NCH:
                            preload(ci + npre)
                        if ci == 2:
                            do_half(0)
                        elif ci == 5:
                            do_half(1)
                        elif ci == 6:
                            do_h_half(0)
                        elif ci == 7:
                            do_h_half(1)

                    # outputs
                    nc.sync.dma_start(out_h[t], h_sb[:].bitcast(fp32))
                    nc.sync.dma_start(out_f[t], f_sb[:])
                    nc.sync.dma_start(out_i[t], i_sb[:])
                    nc.sync.dma_start(out_g[t], g_sb[:])
                    nc.sync.dma_start(out_o[t], o_sb[:])

                    if hT_new is not None:
                        hT_cur = hT_new
                    c_cur = c_new

                nc.sync.dma_start(out_c[:], c_cur[:])

    nc.compile()
    return nc


# gate permutation: packed [f|i|g|o] from reference order [i|f|g|o]
def _gate_perm():
    a = np.arange(H)
    return np.concatenate([H + a, a, 2 * H + a, 3 * H + a])


def _prep_inputs(x, h0, c0, w_ih, w_hh, b_ih, b_hh, t_steps):
    perm = _gate_perm()
    wihT = np.ascontiguousarray(w_ih[perm].T, dtype=np.float32)
    whhT = np.ascontiguousarray(w_hh[perm].T, dtype=np.float32)
    bias = (b_ih + b_hh)[perm].astype(np.float32)
    biasr = np.ascontiguousarray(np.broadcast_to(bias, (128, G4)))
    identity = np.eye(128, dtype=np.float32)

    in_maps = []
    for k in range(NCORES):
        sl = slice(k * BL, (k + 1) * BL)
        xk = x[:t_steps, sl, :]  # [t, BL, I]
        xTk = np.ascontiguousarray(
            xk.transpose(2, 0, 1).reshape(I, t_steps * BL), dtype=np.float32
        )
        h0Tk = np.ascontiguousarray(h0[sl].T, dtype=np.float32)
        c0k = np.ascontiguousarray(c0[sl], dtype=np.float32)
        in_maps.append(
            {
                "xT": xTk,
                "wihT": wihT,
                "whhT": whhT,
                "biasr": biasr,
                "h0T": h0Tk,
                "c0": c0k,
                "ident": identity,
            }
        )
    return in_maps


_NC_CACHE = {}


def run(x, h0, c0, w_ih, w_hh, b_ih, b_hh, t_steps=T, trace=False):
    x = np.asarray(x, dtype=np.float32)
    h0 = np.asarray(h0, dtype=np.float32)
    c0 = np.asarray(c0, dtype=np.float32)
    w_ih = np.asarray(w_ih, dtype=np.float32)
    w_hh = np.asarray(w_hh, dtype=np.float32)
    b_ih = np.asarray(b_ih, dtype=np.float32)
    b_hh = np.asarray(b_hh, dtype=np.float32)

    if t_steps not in _NC_CACHE:
        _NC_CACHE[t_steps] = build_nc(t_steps)
    nc = _NC_CACHE[t_steps]

    in_maps = _prep_inputs(x, h0, c0, w_ih, w_hh, b_ih, b_hh, t_steps)
    res = run_bass_kernel_spmd(
        nc, in_maps, core_ids=list(range(NCORES)), trace=trace
    )

    outs = res.results
    full_h = np.concatenate([outs[k]["out_h"] for k in range(NCORES)], axis=1)
    full_i = np.concatenate([outs[k]["out_i"] for k in range(NCORES)], axis=1)
    full_f = np.concatenate([outs[k]["out_f"] for k in range(NCORES)], axis=1)
    full_g = np.concatenate([outs[k]["out_g"] for k in range(NCORES)], axis=1)
    full_o = np.concatenate([outs[k]["out_o"] for k in range(NCORES)], axis=1)
    c_t = np.concatenate([outs[k]["out_c"] for k in range(NCORES)], axis=0)
    h_t = full_h[-1]
    ret = (full_h, h_t, c_t, full_i, full_f, full_g, full_o)
    return ret, res


def kernel(**inputs):
    ret, _ = run(
        inputs["x"],
        inputs["h0"],
        inputs["c0"],
        inputs["w_ih"],
        inputs["w_hh"],
        inputs["b_ih"],
        inputs["b_hh"],
        t_steps=T,
    )
    return ret


# revision 22
# speedup vs baseline: 10796.2462x; 12.6934x over previous
"""Trainium2 Bass kernel for nn_CustomLSTM (T=512, B=64, I=H=1024).

Strategy: data-parallel over batch (8 cores x 8 rows each). Per core:
  Phase 1: xg = x @ w_ih.T + (b_ih + b_hh)    -- batched matmul, fp32r
  Phase 2: 512-step recurrence:
     gates_t = xg_t + h_{t-1} @ w_hh.T   (PE: identity-selector preload of
               xg into PSUM, then 8 K-tile accumulation matmuls, fp32r)
     i,f,g,o = act(gates)                 (ScalarE, direct from PSUM)
     c = f*c + i*g ; h = o*tanh(c)        (VectorE)
     hT tiles via PE transpose            (stationary operand for next step)

Gate columns are host-permuted to [f | i | g | o] blocks. Weights/x are
host-transposed (pure layout work; all FLOPs run on device).
"""

import os
import sys

sys.path.insert(0, "/opt/trn_rl_repo")

import numpy as np

import concourse.bass as bass
import concourse.mybir as mybir
import concourse.tile as tile
from concourse import bacc
from concourse.bass import ds, ts
from concourse.bass_utils import run_bass_kernel_spmd

fp32 = mybir.dt.float32
fp32r = mybir.dt.float32r
AF = mybir.ActivationFunctionType
ALU = mybir.AluOpType

T, B, I, H = 512, 64, 1024, 1024
NCORES = 8
BL = B // NCORES          # 8 batch rows per core
G4 = 4 * H                # 4096 gate columns
KT = I // 128             # 8 contraction tiles
NCH = G4 // 512           # 8 psum chunks per step
HT = H // 128             # 8 hidden tiles


def build_nc(t_steps: int):
    nc = bacc.Bacc("TRN2", target_bir_lowering=False)

    xT = nc.dram_tensor("xT", [I, t_steps * BL], fp32r, kind="ExternalInput")
    wihT = nc.dram_tensor("wihT", [I, G4], fp32r, kind="ExternalInput")
    whhT = nc.dram_tensor("whhT", [I, G4], fp32r, kind="ExternalInput")
    biasr = nc.dram_tensor("biasr", [128, G4], fp32, kind="ExternalInput")
    h0T = nc.dram_tensor("h0T", [H, BL], fp32r, kind="ExternalInput")
    c0 = nc.dram_tensor("c0", [BL, H], fp32, kind="ExternalInput")
    ident = nc.dram_tensor("ident", [128, 128], fp32r, kind="ExternalInput")

    out_h = nc.dram_tensor("out_h", [t_steps, BL, H], fp32, kind="ExternalOutput")
    out_i = nc.dram_tensor("out_i", [t_steps, BL, H], fp32, kind="ExternalOutput")
    out_f = nc.dram_tensor("out_f", [t_steps, BL, H], fp32, kind="ExternalOutput")
    out_g = nc.dram_tensor("out_g", [t_steps, BL, H], fp32, kind="ExternalOutput")
    out_o = nc.dram_tensor("out_o", [t_steps, BL, H], fp32, kind="ExternalOutput")
    out_c = nc.dram_tensor("out_c", [BL, H], fp32, kind="ExternalOutput")

    n_blocks = (t_steps + 15) // 16

    with tile.TileContext(nc) as tc:
        with (
            tc.tile_pool(name="wpool", bufs=1) as wpool,
            tc.tile_pool(name="cstp", bufs=1) as cstp,
            tc.tile_pool(name="dramp", bufs=1, space="DRAM") as dramp,
        ):
            i128 = cstp.tile([128, 128], fp32r, tag="ident")
            nc.sync.dma_start(i128[:], ident[:])

            xg_dram = dramp.tile([t_steps * BL, G4], fp32r, tag="xg")

            # ---------------- Phase 1: xg = x @ wihT + bias ----------------
            wih_sb = wpool.tile([128, KT, G4], fp32r, tag="wbig", name="wih_sb")
            nc.sync.dma_start(
                wih_sb[:], wihT[:].rearrange("(ko p) g -> p ko g", p=128)
            )
            xT_r = xT[:].rearrange("(ko p) r -> p ko r", p=128)

            with (
                tc.tile_pool(name="p1x", bufs=3) as p1x,
                tc.tile_pool(name="p1b", bufs=1) as p1b,
                tc.tile_pool(name="p1e", bufs=4) as p1e,
                tc.tile_pool(name="ps1", bufs=4, space="PSUM") as ps1,
            ):
                bias_sb = p1b.tile([128, G4], fp32, tag="bias")
                nc.sync.dma_start(bias_sb[:], biasr[:])

                n_mtiles = t_steps * BL // 128
                for m in range(n_mtiles):
                    xt = p1x.tile([128, KT, 128], fp32r, tag="xt")
                    nc.sync.dma_start(xt[:], xT_r[:, :, ds(128 * m, 128)])
                    for n in range(NCH):
                        psx = ps1.tile([128, 512], fp32, tag="px")
                        for ko in range(KT):
                            nc.tensor.matmul(
                                psx[:],
                                xt[:, ko, :],
                                wih_sb[:, ko, ds(512 * n, 512)],
                                start=(ko == 0),
                                stop=(ko == KT - 1),
                            )
                        ev = p1e.tile([128, 512], fp32r, tag="ev")
                        nc.vector.tensor_tensor(
                            ev[:], psx[:], bias_sb[:, ds(512 * n, 512)], ALU.add
                        )
                        nc.sync.dma_start(
                            xg_dram[ds(128 * m, 128), ds(512 * n, 512)], ev[:]
                        )

            # ---------------- Phase 2: recurrence ----------------
            whh_sb = wpool.tile([128, KT, G4], fp32r, tag="wbig", name="whh_sb")
            nc.sync.dma_start(
                whh_sb[:], whhT[:].rearrange("(ko p) g -> p ko g", p=128)
            )

            with (
                tc.tile_pool(name="xgp", bufs=2) as xgp,
                tc.tile_pool(name="htp", bufs=2) as htp,
                tc.tile_pool(name="cp", bufs=2) as cp,
                tc.tile_pool(name="gp", bufs=1) as gp,
                tc.tile_pool(name="hp", bufs=1) as hp,
                tc.tile_pool(name="tcp", bufs=1) as tcp,
                tc.tile_pool(name="psg", bufs=6, space="PSUM") as psg,
                tc.tile_pool(name="pst", bufs=2, space="PSUM") as pst,
            ):
                hT_cur = htp.tile([128, KT, BL], fp32r, tag="hT", name="hT_init")
                nc.sync.dma_start(
                    hT_cur[:], h0T[:].rearrange("(ko p) b -> p ko b", p=128)
                )
                c_cur = cp.tile([BL, H], fp32, tag="c", name="c_init")
                nc.sync.dma_start(c_cur[:], c0[:])

                xg_sb = None
                for t in range(t_steps):
                    blk, s = divmod(t, 16)
                    if s == 0:
                        xg_sb = xgp.tile([128, G4], fp32r, tag="xg", name=f"xgb_{blk}")
                        nrows = min(128, t_steps * BL - 128 * blk)
                        nc.sync.dma_start(
                            xg_sb[:nrows], xg_dram[ds(128 * blk, nrows), :]
                        )

                    # gate tiles for this step (packed order f,i,g,o)
                    f_sb = gp.tile([BL, H], fp32, tag="fsb")
                    i_sb = gp.tile([BL, H], fp32, tag="isb")
                    g_sb = gp.tile([BL, H], fp32, tag="gsb")
                    o_sb = gp.tile([BL, H], fp32, tag="osb")
                    c_new = cp.tile([BL, H], fp32, tag="c")
                    tmp = tcp.tile([BL, H], fp32, tag="tmp")
                    tc_sb = tcp.tile([BL, H], fp32, tag="tc")
                    h_sb = hp.tile([BL, H], fp32r, tag="h")
                    hT_new = (
                        htp.tile([128, KT, BL], fp32r, tag="hT", name=f"hT_{t}")
                        if t < t_steps - 1
                        else None
                    )

                    # chunk order: f0 i0 g0 f1 i1 g1 o0 o1 -> half hh of each
                    # gate block; elementwise runs per-half as soon as its
                    # gates are ready, so the h-tail overlaps the o-matmuls
                    # and next-step preloads.
                    CHORD = [(0, 0), (1, 0), (2, 0), (0, 1), (1, 1), (2, 1), (3, 0), (3, 1)]
                    gtiles = [f_sb, i_sb, g_sb, o_sb]
                    gfuncs = [AF.Sigmoid, AF.Sigmoid, AF.Tanh, AF.Sigmoid]

                    pgs = [psg.tile([BL, 512], fp32, tag="pg", name=f"pg_{t}_{k}") for k in range(NCH)]
                    npre = 6

                    def preload(ci):
                        q, hh = CHORD[ci]
                        nc.tensor.matmul(
                            pgs[ci][:],
                            i128[:, ds(8 * s, BL)],
                            xg_sb[:, ds(512 * (2 * q + hh), 512)],
                            start=True,
                            stop=False,
                        )

                    def do_half(hh):
                        # c/h update for hidden cols [512*hh : 512*hh+512]
                        cs = ds(512 * hh, 512)
                        nc.vector.tensor_tensor(tmp[:, cs], i_sb[:, cs], g_sb[:, cs], ALU.mult)
                        nc.vector.tensor_tensor(c_new[:, cs], f_sb[:, cs], c_cur[:, cs], ALU.mult)
                        nc.vector.tensor_tensor(c_new[:, cs], c_new[:, cs], tmp[:, cs], ALU.add)
                        nc.scalar.activation(tc_sb[:, cs], c_new[:, cs], AF.Tanh)

                    pt128 = (
                        pst.tile([128, KT * BL], fp32r, tag="pt", name=f"pt_{t}")
                        if hT_new is not None
                        else None
                    )

                    def do_h_half(hh):
                        cs = ds(512 * hh, 512)
                        nc.vector.tensor_tensor(h_sb[:, cs], o_sb[:, cs], tc_sb[:, cs], ALU.mult)
                        if hT_new is not None:
                            for j in range(4 * hh, 4 * hh + 4):
                                nc.tensor.transpose(
                                    pt128[:, ds(BL * j, BL)],
                                    h_sb[:, ds(128 * j, 128)],
                                    i128[:BL, :BL],
                                )
                            nc.vector.tensor_copy(
                                hT_new[:, ds(4 * hh, 4), :],
                                pt128[:, ds(32 * hh, 32)].rearrange(
                                    "p (j b) -> p j b", b=BL
                                ),
                            )

                    for ci in range(npre):
                        preload(ci)
                    for ci in range(NCH):
                        q, hh = CHORD[ci]
                        pg = pgs[ci]
                        for ko in range(KT):
                            nc.tensor.matmul(
                                pg[:],
                                hT_cur[:, ko, :],
                                whh_sb[:, ko, ds(512 * (2 * q + hh), 512)],
                                start=False,
                                stop=(ko == KT - 1),
                            )
                        nc.scalar.activation(
                            gtiles[q][:, ds(512 * hh, 512)], pg[:], gfuncs[q]
                        )
                        if ci + npre < NCH:
                            preload(ci + npre)
                        if ci == 2:
                            do_half(0)
                        elif ci == 5:
                            do_half(1)
                        elif ci == 6:
                            do_h_half(0)
                        elif ci == 7:
                            do_h_half(1)

                    # outputs
                    nc.sync.dma_start(out_h[t], h_sb[:].bitcast(fp32))
                    nc.sync.dma_start(out_f[t], f_sb[:])
                    nc.sync.dma_start(out_i[t], i_sb[:])
                    nc.sync.dma_start(out_g[t], g_sb[:])
                    nc.sync.dma_start(out_o[t], o_sb[:])

                    if hT_new is not None:
                        hT_cur = hT_new
                    c_cur = c_new

                nc.sync.dma_start(out_c[:], c_cur[:])

    nc.compile()
    return nc


# gate permutation: packed [f|i|g|o] from reference order [i|f|g|o]
def _gate_perm():
    a = np.arange(H)
    return np.concatenate([H + a, a, 2 * H + a, 3 * H + a])


def _prep_inputs(x, h0, c0, w_ih, w_hh, b_ih, b_hh, t_steps):
    perm = _gate_perm()
    wihT = np.ascontiguousarray(w_ih[perm].T, dtype=np.float32)
    whhT = np.ascontiguousarray(w_hh[perm].T, dtype=np.float32)
    bias = (b_ih + b_hh)[perm].astype(np.float32)
    biasr = np.ascontiguousarray(np.broadcast_to(bias, (128, G4)))
    identity = np.eye(128, dtype=np.float32)

    in_maps = []
    for k in range(NCORES):
        sl = slice(k * BL, (k + 1) * BL)
        xk = x[:t_steps, sl, :]  # [t, BL, I]
        xTk = np.ascontiguousarray(
            xk.transpose(2, 0, 1).reshape(I, t_steps * BL), dtype=np.float32
        )
        h0Tk = np.ascontiguousarray(h0[sl].T, dtype=np.float32)
        c0k = np.ascontiguousarray(c0[sl], dtype=np.float32)
        in_maps.append(
            {
                "xT": xTk,
                "wihT": wihT,
                "whhT": whhT,
                "biasr": biasr,
                "h0T": h0Tk,
                "c0": c0k,
                "ident": identity,
            }
        )
    return in_maps


_NC_CACHE = {}


def run(x, h0, c0, w_ih, w_hh, b_ih, b_hh, t_steps=T, trace=False):
    x = np.asarray(x, dtype=np.float32)
    h0 = np.asarray(h0, dtype=np.float32)
    c0 = np.asarray(c0, dtype=np.float32)
    w_ih = np.asarray(w_ih, dtype=np.float32)
    w_hh = np.asarray(w_hh, dtype=np.float32)
    b_ih = np.asarray(b_ih, dtype=np.float32)
    b_hh = np.asarray(b_hh, dtype=np.float32)

    if t_steps not in _NC_CACHE:
        _NC_CACHE[t_steps] = build_nc(t_steps)
    nc = _NC_CACHE[t_steps]

    in_maps = _prep_inputs(x, h0, c0, w_ih, w_hh, b_ih, b_hh, t_steps)
    res = run_bass_kernel_spmd(
        nc, in_maps, core_ids=list(range(NCORES)), trace=trace
    )

    outs = res.results
    full_h = np.concatenate([outs[k]["out_h"] for k in range(NCORES)], axis=1)
    full_i = np.concatenate([outs[k]["out_i"] for k in range(NCORES)], axis=1)
    full_f = np.concatenate([outs[k]["out_f"] for k in range(NCORES)], axis=1)
    full_g = np.concatenate([outs[k]["out_g"] for k in range(NCORES)], axis=1)
    full_o = np.concatenate([outs[k]["out_o"] for k in range(NCORES)], axis=1)
    c_t = np.concatenate([outs[k]["out_c"] for k in range(NCORES)], axis=0)
    h_t = full_h[-1]
    ret = (full_h, h_t, c_t, full_i, full_f, full_g, full_o)
    return ret, res


def kernel(**inputs):
    ret, _ = run(
        inputs["x"],
        inputs["h0"],
        inputs["c0"],
        inputs["w_ih"],
        inputs["w_hh"],
        inputs["b_ih"],
        inputs["b_hh"],
        t_steps=T,
    )
    return ret


# revision 27
# speedup vs baseline: 11159.9273x; 1.0337x over previous
"""Trainium2 Bass kernel for nn_CustomLSTM (T=512, B=64, I=H=1024).

Strategy: data-parallel over batch (8 cores x 8 rows each). Per core:
  Phase 1: xg = x @ w_ih.T + (b_ih + b_hh)    -- batched matmul, fp32r
  Phase 2: 512-step recurrence:
     gates_t = xg_t + h_{t-1} @ w_hh.T   (PE: identity-selector preload of
               xg into PSUM, then 8 K-tile accumulation matmuls, fp32r)
     i,f,g,o = act(gates)                 (ScalarE, direct from PSUM)
     c = f*c + i*g ; h = o*tanh(c)        (VectorE)
     hT tiles via PE transpose            (stationary operand for next step)

Gate columns are host-permuted to [f | i | g | o] blocks. Weights/x are
host-transposed (pure layout work; all FLOPs run on device).
"""

import os
import sys

sys.path.insert(0, "/opt/trn_rl_repo")

import numpy as np

import concourse.bass as bass
import concourse.mybir as mybir
import concourse.tile as tile
from concourse import bacc
from concourse.bass import ds, ts
from concourse.bass_utils import run_bass_kernel_spmd

fp32 = mybir.dt.float32
fp32r = mybir.dt.float32r
AF = mybir.ActivationFunctionType
ALU = mybir.AluOpType

T, B, I, H = 512, 64, 1024, 1024
NCORES = 8
BL = B // NCORES          # 8 batch rows per core
G4 = 4 * H                # 4096 gate columns
KT = I // 128             # 8 contraction tiles
NCH = G4 // 512           # 8 psum chunks per step
HT = H // 128             # 8 hidden tiles


def build_nc(t_steps: int):
    nc = bacc.Bacc("TRN2", target_bir_lowering=False)

    xT = nc.dram_tensor("xT", [I, t_steps * BL], fp32r, kind="ExternalInput")
    wihT = nc.dram_tensor("wihT", [I, G4], fp32r, kind="ExternalInput")
    whhT = nc.dram_tensor("whhT", [I, G4], fp32r, kind="ExternalInput")
    biasr = nc.dram_tensor("biasr", [128, G4], fp32, kind="ExternalInput")
    h0T = nc.dram_tensor("h0T", [H, BL], fp32r, kind="ExternalInput")
    c0 = nc.dram_tensor("c0", [BL, H], fp32, kind="ExternalInput")
    ident = nc.dram_tensor("ident", [128, 128], fp32r, kind="ExternalInput")
    zeros2m = nc.dram_tensor("zeros2m", [128, G4], fp32r, kind="ExternalInput")

    out_h = nc.dram_tensor("out_h", [t_steps, BL, H], fp32, kind="ExternalOutput")
    out_i = nc.dram_tensor("out_i", [t_steps, BL, H], fp32, kind="ExternalOutput")
    out_f = nc.dram_tensor("out_f", [t_steps, BL, H], fp32, kind="ExternalOutput")
    out_g = nc.dram_tensor("out_g", [t_steps, BL, H], fp32, kind="ExternalOutput")
    out_o = nc.dram_tensor("out_o", [t_steps, BL, H], fp32, kind="ExternalOutput")
    out_c = nc.dram_tensor("out_c", [BL, H], fp32, kind="ExternalOutput")

    n_blocks = (t_steps + 15) // 16

    with tile.TileContext(nc) as tc:
        with (
            tc.tile_pool(name="wpool", bufs=1) as wpool,
            tc.tile_pool(name="cstp", bufs=1) as cstp,
            tc.tile_pool(name="dramp", bufs=1, space="DRAM") as dramp,
        ):
            i128 = cstp.tile([128, 128], fp32r, tag="ident")
            nc.sync.dma_start(i128[:], ident[:])

            xg_dram = dramp.tile([t_steps * BL, G4], fp32r, tag="xg")

            # ---------------- Phase 1: xg = x @ wihT + bias ----------------
            wih_sb = wpool.tile([128, KT, G4], fp32r, tag="wbig", name="wih_sb")
            nc.sync.dma_start(
                wih_sb[:], wihT[:].rearrange("(ko p) g -> p ko g", p=128)
            )
            xT_r = xT[:].rearrange("(ko p) r -> p ko r", p=128)

            with (
                tc.tile_pool(name="p1x", bufs=3) as p1x,
                tc.tile_pool(name="p1b", bufs=1) as p1b,
                tc.tile_pool(name="p1e", bufs=4) as p1e,
                tc.tile_pool(name="ps1", bufs=4, space="PSUM") as ps1,
            ):
                bias_sb = p1b.tile([128, G4], fp32, tag="bias")
                nc.sync.dma_start(bias_sb[:], biasr[:])

                n_mtiles = t_steps * BL // 128
                for m in range(n_mtiles):
                    xt = p1x.tile([128, KT, 128], fp32r, tag="xt")
                    nc.sync.dma_start(xt[:], xT_r[:, :, ds(128 * m, 128)])
                    for n in range(NCH):
                        psx = ps1.tile([128, 512], fp32, tag="px")
                        for ko in range(KT):
                            nc.tensor.matmul(
                                psx[:],
                                xt[:, ko, :],
                                wih_sb[:, ko, ds(512 * n, 512)],
                                start=(ko == 0),
                                stop=(ko == KT - 1),
                            )
                        ev = p1e.tile([128, 512], fp32r, tag="ev")
                        nc.vector.tensor_tensor(
                            ev[:], psx[:], bias_sb[:, ds(512 * n, 512)], ALU.add
                        )
                        nc.sync.dma_start(
                            xg_dram[ds(128 * m, 128), ds(512 * n, 512)], ev[:]
                        )

            # ---------------- Phase 2: recurrence ----------------
            whh_sb = wpool.tile([128, KT, G4], fp32r, tag="wbig", name="whh_sb")
            nc.sync.dma_start(
                whh_sb[:], whhT[:].rearrange("(ko p) g -> p ko g", p=128)
            )

            with (
                tc.tile_pool(name="xgp", bufs=1) as xgp,
                tc.tile_pool(name="htp", bufs=2) as htp,
                tc.tile_pool(name="cp", bufs=2) as cp,
                tc.tile_pool(name="gp", bufs=1) as gp,
                tc.tile_pool(name="hp", bufs=1) as hp,
                tc.tile_pool(name="tcp", bufs=1) as tcp,
                tc.tile_pool(name="gsp", bufs=2) as gsp,
                tc.tile_pool(name="psg", bufs=6, space="PSUM") as psg,
                tc.tile_pool(name="pst", bufs=2, space="PSUM") as pst,
            ):
                hT_cur = htp.tile([128, KT, BL], fp32r, tag="hT", name="hT_init")
                nc.sync.dma_start(
                    hT_cur[:], h0T[:].rearrange("(ko p) b -> p ko b", p=128)
                )
                c_cur = cp.tile([BL, H], fp32, tag="c", name="c_init")
                nc.sync.dma_start(c_cur[:], c0[:])

                # Two persistent xg buffers; 4 steps per [128, G4] block with
                # step u at partition offset 32*u (32-aligned so VectorE may
                # read per-step rows directly). Rows 8:32 of each group are
                # zeroed once and never rewritten: the selector preload
                # contracts all 128 partitions, and 0 x garbage could be NaN.
                xg_bufs = [
                    xgp.tile([128, G4], fp32r, tag="xga", name="xg_bufA"),
                    xgp.tile([128, G4], fp32r, tag="xgb", name="xg_bufB"),
                ]
                nc.sync.dma_start(xg_bufs[0][:], zeros2m[:])
                nc.sync.dma_start(xg_bufs[1][:], zeros2m[:])

                xg_sb = None
                for t in range(t_steps):
                    blk, s4 = divmod(t, 4)
                    if s4 == 0:
                        xg_sb = xg_bufs[blk % 2]
                        for u in range(4):
                            if 4 * blk + u >= t_steps:
                                break
                            nc.sync.dma_start(
                                xg_sb[ds(32 * u, BL), :],
                                xg_dram[ds(32 * blk + BL * u, BL), :],
                            )

                    # gate tiles for this step (packed order f,i,g,o)
                    f_sb = gp.tile([BL, H], fp32, tag="fsb")
                    i_sb = gp.tile([BL, H], fp32, tag="isb")
                    g_sb = gp.tile([BL, H], fp32, tag="gsb")
                    o_sb = gp.tile([BL, H], fp32, tag="osb")
                    c_new = cp.tile([BL, H], fp32, tag="c")
                    tmp = tcp.tile([BL, H], fp32, tag="tmp")
                    tc_sb = tcp.tile([BL, H], fp32, tag="tc")
                    h_sb = hp.tile([BL, H], fp32r, tag="h")
                    hT_new = (
                        htp.tile([128, KT, BL], fp32r, tag="hT", name=f"hT_{t}")
                        if t < t_steps - 1
                        else None
                    )

                    # chunk order: f0 i0 g0 f1 i1 g1 o0 o1 -> half hh of each
                    # gate block; elementwise runs per-half as soon as its
                    # gates are ready, so the h-tail overlaps the o-matmuls
                    # and next-step preloads.
                    CHORD = [(0, 0), (1, 0), (2, 0), (0, 1), (1, 1), (2, 1), (3, 0), (3, 1)]
                    gtiles = [f_sb, i_sb, g_sb, o_sb]
                    gfuncs = [AF.Sigmoid, AF.Sigmoid, AF.Tanh, AF.Sigmoid]

                    pgs = [psg.tile([BL, 512], fp32, tag="pg", name=f"pg_{t}_{k}") for k in range(NCH)]
                    npre = 6
                    # chunks whose xg add runs on VectorE (PSUM + xg -> gs)
                    # instead of a PE selector preload; picked so the adds
                    # hide under later matmuls (consumed at ci==5)
                    DVE_ADD = {0, 1, 3, 4}

                    def preload(ci):
                        if ci in DVE_ADD:
                            return
                        q, hh = CHORD[ci]
                        nc.tensor.matmul(
                            pgs[ci][:],
                            i128[:, ds(32 * s4, BL)],
                            xg_sb[:, ds(512 * (2 * q + hh), 512)],
                            start=True,
                            stop=False,
                        )

                    def do_half(hh):
                        # c/h update for hidden cols [512*hh : 512*hh+512]
                        cs = ds(512 * hh, 512)
                        nc.vector.tensor_tensor(tmp[:, cs], i_sb[:, cs], g_sb[:, cs], ALU.mult)
                        nc.vector.tensor_tensor(c_new[:, cs], f_sb[:, cs], c_cur[:, cs], ALU.mult)
                        nc.vector.tensor_tensor(c_new[:, cs], c_new[:, cs], tmp[:, cs], ALU.add)
                        nc.scalar.activation(tc_sb[:, cs], c_new[:, cs], AF.Tanh)

                    pt128 = (
                        pst.tile([128, KT * BL], fp32r, tag="pt", name=f"pt_{t}")
                        if hT_new is not None
                        else None
                    )

                    def do_h_half(hh):
                        cs = ds(512 * hh, 512)
                        nc.vector.tensor_tensor(h_sb[:, cs], o_sb[:, cs], tc_sb[:, cs], ALU.mult)
                        if hT_new is not None:
                            for j in range(4 * hh, 4 * hh + 4):
                                nc.tensor.transpose(
                                    pt128[:, ds(BL * j, BL)],
                                    h_sb[:, ds(128 * j, 128)],
                                    i128[:BL, :BL],
                                )
                            nc.vector.tensor_copy(
                                hT_new[:, ds(4 * hh, 4), :],
                                pt128[:, ds(32 * hh, 32)].rearrange(
                                    "p (j b) -> p j b", b=BL
                                ),
                            )

                    for ci in range(npre):
                        preload(ci)
                    for ci in range(NCH):
                        q, hh = CHORD[ci]
                        pg = pgs[ci]
                        for ko in range(KT):
                            nc.tensor.matmul(
                                pg[:],
                                hT_cur[:, ko, :],
                                whh_sb[:, ko, ds(512 * (2 * q + hh), 512)],
                                start=(ko == 0 and ci in DVE_ADD),
                                stop=(ko == KT - 1),
                            )
                        if ci in DVE_ADD:
                            gs = gsp.tile([BL, 512], fp32, tag="gs", name=f"gs_{t}_{ci}")
                            nc.vector.tensor_tensor(
                                gs[:],
                                pg[:],
                                xg_sb[
                                    ds(32 * s4, BL), ds(512 * (2 * q + hh), 512)
                                ].bitcast(fp32),
                                ALU.add,
                            )
                            nc.scalar.activation(
                                gtiles[q][:, ds(512 * hh, 512)], gs[:], gfuncs[q]
                            )
                        else:
                            nc.scalar.activation(
                                gtiles[q][:, ds(512 * hh, 512)], pg[:], gfuncs[q]
                            )
                        if ci + npre < NCH:
                            preload(ci + npre)
                        if ci == 2:
                            do_half(0)
                        elif ci == 5:
                            do_half(1)
                        elif ci == 6:
                            do_h_half(0)
                        elif ci == 7:
                            do_h_half(1)

                    # outputs
                    nc.sync.dma_start(out_h[t], h_sb[:].bitcast(fp32))
                    nc.sync.dma_start(out_f[t], f_sb[:])
                    nc.sync.dma_start(out_i[t], i_sb[:])
                    nc.sync.dma_start(out_g[t], g_sb[:])
                    nc.sync.dma_start(out_o[t], o_sb[:])

                    if hT_new is not None:
                        hT_cur = hT_new
                    c_cur = c_new

                nc.sync.dma_start(out_c[:], c_cur[:])

    nc.compile()
    return nc


# gate permutation: packed [f|i|g|o] from reference order [i|f|g|o]
def _gate_perm():
    a = np.arange(H)
    return np.concatenate([H + a, a, 2 * H + a, 3 * H + a])


def _prep_inputs(x, h0, c0, w_ih, w_hh, b_ih, b_hh, t_steps):
    perm = _gate_perm()
    wihT = np.ascontiguousarray(w_ih[perm].T, dtype=np.float32)
    whhT = np.ascontiguousarray(w_hh[perm].T, dtype=np.float32)
    bias = (b_ih + b_hh)[perm].astype(np.float32)
    biasr = np.ascontiguousarray(np.broadcast_to(bias, (128, G4)))
    identity = np.eye(128, dtype=np.float32)

    in_maps = []
    for k in range(NCORES):
        sl = slice(k * BL, (k + 1) * BL)
        xk = x[:t_steps, sl, :]  # [t, BL, I]
        xTk = np.ascontiguousarray(
            xk.transpose(2, 0, 1).reshape(I, t_steps * BL), dtype=np.float32
        )
        h0Tk = np.ascontiguousarray(h0[sl].T, dtype=np.float32)
        c0k = np.ascontiguousarray(c0[sl], dtype=np.float32)
        in_maps.append(
            {
                "xT": xTk,
                "wihT": wihT,
                "whhT": whhT,
                "biasr": biasr,
                "h0T": h0Tk,
                "c0": c0k,
                "ident": identity,
                "zeros2m": np.zeros((128, G4), np.float32),
            }
        )
    return in_maps


_NC_CACHE = {}


def run(x, h0, c0, w_ih, w_hh, b_ih, b_hh, t_steps=T, trace=False):
    x = np.asarray(x, dtype=np.float32)
    h0 = np.asarray(h0, dtype=np.float32)
    c0 = np.asarray(c0, dtype=np.float32)
    w_ih = np.asarray(w_ih, dtype=np.float32)
    w_hh = np.asarray(w_hh, dtype=np.float32)
    b_ih = np.asarray(b_ih, dtype=np.float32)
    b_hh = np.asarray(b_hh, dtype=np.float32)

    if t_steps not in _NC_CACHE:
        _NC_CACHE[t_steps] = build_nc(t_steps)
    nc = _NC_CACHE[t_steps]

    in_maps = _prep_inputs(x, h0, c0, w_ih, w_hh, b_ih, b_hh, t_steps)
    res = run_bass_kernel_spmd(
        nc, in_maps, core_ids=list(range(NCORES)), trace=trace
    )

    outs = res.results
    full_h = np.concatenate([outs[k]["out_h"] for k in range(NCORES)], axis=1)
    full_i = np.concatenate([outs[k]["out_i"] for k in range(NCORES)], axis=1)
    full_f = np.concatenate([outs[k]["out_f"] for k in range(NCORES)], axis=1)
    full_g = np.concatenate([outs[k]["out_g"] for k in range(NCORES)], axis=1)
    full_o = np.concatenate([outs[k]["out_o"] for k in range(NCORES)], axis=1)
    c_t = np.concatenate([outs[k]["out_c"] for k in range(NCORES)], axis=0)
    h_t = full_h[-1]
    ret = (full_h, h_t, c_t, full_i, full_f, full_g, full_o)
    return ret, res


def kernel(**inputs):
    ret, _ = run(
        inputs["x"],
        inputs["h0"],
        inputs["c0"],
        inputs["w_ih"],
        inputs["w_hh"],
        inputs["b_ih"],
        inputs["b_hh"],
        t_steps=T,
    )
    return ret
